# revision 1
# baseline (speedup 1.0000x reference)
"""Backward 2x2 average pooling (stride 2) == 2x nearest-neighbor upsample
scaled by the kernel taps:

    out[b, 2i+di, 2j+dj, c] = kernel[di, dj, 0, 0] * x[b, i, j, c]

x: (32, 112, 112, 128) f32, kernel: (2, 2, 1, 1) f32 -> out: (32, 224, 224, 128).

Pure data-parallel across 8 NeuronCores: 4 batch images per core.
Per core, x is viewed as (448, 14336) rows ((b,h) major, w*c contiguous) and
out as (896, 28672).  Output row 2r+di is input row r with every 128-float
channel block duplicated (dj) and scaled.  The W-duplication+scale happens
on-chip (DVE tensor_scalar), the H-duplication is done by storing the same
SBUF tile to two interleaved HBM row sets, keeping every DMA large and fully
contiguous per partition.

Raw Bass (no Tile): this toolchain's walrus rejects instructions carrying
more than one sync-wait, so synchronization is done with explicit standalone
wait_ge instructions and per-buffer-slot semaphores (one per in/out ring
slot, plus one DVE counter), each instruction carrying at most one sem
event.  All DMAs are SWDGE (gpsimd): HWDGE DMAs from raw bass crash this
hardware (NRT_EXEC_UNIT_UNRECOVERABLE).

Grading entrypoint: kernel(x, kernel) -> (32, 224, 224, 128) float32.
"""

import numpy as np

import concourse.bass as bass
import concourse.mybir as mybir
from concourse import bass_utils

N_CORES = 8
B, HP, WP, C = 32, 112, 112, 128
BPC = B // N_CORES            # batch images per core
ROWS = BPC * HP               # 448 input rows per core
INF = WP * C                  # 14336 floats per input row
OUTF = 2 * INF                # 28672 floats per output row

FP32 = mybir.dt.float32

IN_BUFS = 4                   # input-tile ring slots (load prefetch depth)
OUT_BUFS = 3                  # output-tile ring slots (per di in general path)


def _build(scales, rows=ROWS, wp=WP, fch=4, in_bufs=IN_BUFS, out_bufs=OUT_BUFS):
    """scales: ((k00,k01),(k10,k11)) python floats.

    Builds the per-core module: x (rows, wp*C) -> out (2*rows, 2*wp*C).
    """
    inf = wp * C
    f = inf // fch                      # input floats per chunk
    assert f % C == 0
    uniform = scales[0][0] == scales[0][1] == scales[1][0] == scales[1][1]

    # iteration space: (row chunk, free chunk)
    chunks = []
    for s in range(0, rows, 128):
        p = min(128, rows - s)
        for fc in range(fch):
            chunks.append((s, p, fc * f))
    niter = len(chunks)

    nc = bass.Bass()
    x = nc.dram_tensor("x", (rows, inf), FP32, kind="ExternalInput")
    out = nc.dram_tensor("out", (2 * rows, 2 * inf), FP32, kind="ExternalOutput")
    out3 = out[:, :].rearrange("(r two) f -> r two f", two=2)

    from contextlib import ExitStack
    with ExitStack() as ctx:
        n_di = 1 if uniform else 2
        obufs = out_bufs if uniform else 2
        # Per-slot DMA-completion sems: a slot has at most one DMA cycle in
        # flight, so fixed per-cycle thresholds are race-free (completions of
        # different DMAs on one queue are not ordered, so one cumulative sem
        # would be).
        load_sems = [
            ctx.enter_context(nc.semaphore(f"load_sem{i}"))
            for i in range(in_bufs)
        ]
        store_sems = [
            ctx.enter_context(nc.semaphore(f"store_sem{i}"))
            for i in range(obufs)
        ]
        mul_sem = ctx.enter_context(nc.semaphore("mul_sem"))
        in_tiles = [
            ctx.enter_context(nc.sbuf_tensor(f"in_tile{i}", [128, f], FP32))
            for i in range(in_bufs)
        ]
        out_tiles = [
            [ctx.enter_context(
                nc.sbuf_tensor(f"out_tile{d}_{i}", [128, 2 * f], FP32))
             for i in range(obufs)]
            for d in range(n_di)
        ]
        muls_per_iter = 2 * n_di

        with nc.Block() as b0:
            @b0.gpsimd
            def _(g):
                for sem in (*load_sems, *store_sems, mul_sem):
                    g.sem_clear(sem)

        with nc.Block() as blk:
            # All DMAs on the Pool engine (SWDGE).  HWDGE (SP/ACT) variants
            # simulate fine but hit NRT_EXEC_UNIT_UNRECOVERABLE on this
            # hardware, so SWDGE it is.
            @blk.gpsimd
            def _(g):
                def load(t):
                    s, p, fo = chunks[t]
                    g.dma_start(
                        in_tiles[t % in_bufs][:p], x[s:s + p, fo:fo + f]
                    ).then_inc(load_sems[t % in_bufs], 16)

                for t in range(min(in_bufs, niter)):
                    load(t)
                for t in range(niter):
                    s, p, fo = chunks[t]
                    # both output rows of iter t depend on all muls of iter t
                    g.wait_ge(mul_sem, muls_per_iter * (t + 1))
                    for di in range(2):
                        src = out_tiles[di % n_di][t % obufs]
                        g.dma_start(
                            out3[s:s + p, di, 2 * fo:2 * fo + 2 * f],
                            src[:p],
                        ).then_inc(store_sems[t % obufs], 16)
                    if t + in_bufs < niter:
                        # in-slot WAR vs muls of iter t is the same wait we
                        # just did; no extra wait needed.
                        load(t + in_bufs)

            @blk.vector
            def _(v):
                for t in range(niter):
                    s, p, fo = chunks[t]
                    v.wait_ge(load_sems[t % in_bufs],
                              16 * (t // in_bufs + 1))
                    if t >= obufs:
                        # out-slot WAR: both stores of iter t-obufs done
                        v.wait_ge(store_sems[t % obufs],
                                  32 * (t // obufs))
                    in3 = in_tiles[t % in_bufs][:p].rearrange(
                        "p (j c) -> p j c", c=C)
                    for di in range(n_di):
                        o4 = out_tiles[di][t % obufs][:p].rearrange(
                            "p (j two c) -> p j two c", two=2, c=C)
                        v.tensor_scalar_mul(
                            o4[:, :, 0], in3, scales[di][0]
                        ).then_inc(mul_sem, 1)
                        v.tensor_scalar_mul(
                            o4[:, :, 1], in3, scales[di][1]
                        ).then_inc(mul_sem, 1)
    return nc


_nc_cache = {}


def _get_nc(scales):
    if scales not in _nc_cache:
        _nc_cache[scales] = _build(scales)
    return _nc_cache[scales]


def _scales_of(kernel):
    return ((float(kernel[0, 0, 0, 0]), float(kernel[0, 1, 0, 0])),
            (float(kernel[1, 0, 0, 0]), float(kernel[1, 1, 0, 0])))


def _run(x, kernel, **run_kwargs):
    nc = _get_nc(_scales_of(kernel))
    x = np.ascontiguousarray(x, dtype=np.float32)
    in_maps = [
        {"x": x[c * BPC:(c + 1) * BPC].reshape(ROWS, INF)}
        for c in range(N_CORES)
    ]
    res = bass_utils.run_bass_kernel_spmd(
        nc, in_maps, core_ids=list(range(N_CORES)), **run_kwargs)
    outs = [
        r["out"].reshape(BPC, 2 * HP, 2 * WP, C) for r in res.results
    ]
    return np.concatenate(outs, axis=0), res


_exec_cache = {}


def _run_fast(x, kernel):
    """Same execution as _run (run_bass_kernel_spmd's axon redirect builds
    this exact shard_map jit), but the jit is built once per scales and
    cached, so repeated calls skip the ~40 s re-lowering/recompile."""
    scales = _scales_of(kernel)
    if scales not in _exec_cache:
        _exec_cache[scales] = _make_sharded(_get_nc(scales))
    sharded, _ = _exec_cache[scales]
    x = np.ascontiguousarray(x, dtype=np.float32)
    xc = x.reshape(N_CORES * ROWS, INF)
    z = np.zeros((N_CORES * 2 * ROWS, OUTF), np.float32)
    (o,) = sharded(xc, z)
    return np.asarray(o).reshape(B, 2 * HP, 2 * WP, C)


def kernel(x, kernel):
    try:
        return _run_fast(x, kernel)
    except Exception:
        out, _ = _run(x, kernel)
        return out


# ---------------------------------------------------------------------------
# test-only helpers (not used by the grading path)

_UNIFORM = ((0.25, 0.25), (0.25, 0.25))


def _simulate(scales=_UNIFORM):
    from concourse.timeline_sim import TimelineSim
    nc = _get_nc(scales)
    return TimelineSim(nc).simulate()


def _coresim_check(scales=_UNIFORM, rows=16, wp=8, fch=2, seed=0):
    """Functional + race check of a miniature config in CoreSim."""
    from concourse.bass_interp import CoreSim
    rng = np.random.default_rng(seed)
    inf = wp * C
    nc = _build(scales, rows=rows, wp=wp, fch=fch)
    x = rng.standard_normal((rows, inf), dtype=np.float32)
    sim = CoreSim(nc)
    sim.assign_tensors({"x": x})
    sim.simulate()
    got = np.array(sim.tensor("out"))          # (2*rows, 2*inf)
    k = np.array(scales, np.float32)           # (2,2)
    xb = x.reshape(rows, wp, C)
    exp = np.empty((rows, 2, wp, 2, C), np.float32)
    for di in range(2):
        for dj in range(2):
            exp[:, di, :, dj, :] = xb * k[di, dj]
    exp = exp.reshape(2 * rows, 2 * inf)
    err = float(np.abs(got - exp).max())
    return err, sim.time


def _make_sharded(nc, donate=True):
    """Mirror bass2jax.run_bass_via_pjrt's multi-core path, but with the jit
    built once so buffers stay device-resident across repeated timed calls.
    The module carries a hidden partition_id ExternalInput which must be fed
    via PartitionIdOp, exactly as run_bass_via_pjrt does."""
    import jax
    from jax.experimental.shard_map import shard_map
    from jax.sharding import Mesh, NamedSharding, PartitionSpec
    from concourse import bass2jax

    bass2jax.install_neuronx_cc_hook()
    out_aval = jax.core.ShapedArray((2 * ROWS, OUTF), np.float32)
    partition_name = nc.partition_id_tensor.name

    def _body(x_in, out_zero):
        outs = bass2jax._bass_exec_p.bind(
            x_in, out_zero, bass2jax.partition_id_tensor(),
            out_avals=(out_aval,),
            in_names=("x", "out", partition_name),
            out_names=("out",),
            lowering_input_output_aliases=(),
            sim_require_finite=True,
            sim_require_nnan=True,
            nc=nc,
        )
        return tuple(outs)

    devices = jax.devices()[:N_CORES]
    mesh = Mesh(np.asarray(devices), ("core",))
    sharded = jax.jit(
        shard_map(_body, mesh=mesh,
                  in_specs=(PartitionSpec("core"), PartitionSpec("core")),
                  out_specs=(PartitionSpec("core"),), check_rep=False),
        donate_argnums=(1,) if donate else (), keep_unused=True,
    )
    shard = NamedSharding(mesh, PartitionSpec("core"))
    return sharded, shard


def _bench(x, kernel, n_per_batch=(10, 50, 100)):
    """Chained device-resident executions: each call donates the previous
    call's output buffer, so the timed loop never touches the host.  Returns
    {N: seconds} for each batch size; the slope between batch sizes bounds
    per-execution time (dispatch overhead through the axon relay included)."""
    import time
    import jax
    nc = _get_nc(_scales_of(kernel))
    sharded, shard = _make_sharded(nc)

    x = np.ascontiguousarray(x, dtype=np.float32)
    x_dev = jax.device_put(x.reshape(N_CORES * ROWS, INF), shard)
    o = jax.device_put(np.zeros((N_CORES * 2 * ROWS, OUTF), np.float32), shard)

    (o,) = sharded(x_dev, o)          # warmup / compile
    o.block_until_ready()

    res = {}
    for n in n_per_batch:
        t0 = time.perf_counter()
        for _ in range(n):
            (o,) = sharded(x_dev, o)
        o.block_until_ready()
        res[n] = time.perf_counter() - t0
    return res

